# revision 5
# baseline (speedup 1.0000x reference)
"""Trainium2 Bass kernel for nn_CenterDistLoss (segment_reduce).

Strategy (data-parallel over batch, 4 batches per core on 8 cores):
  Per-pixel labels lab = round(y_pr)*mask are never materialized; instead the
  kernel streams CUMULATIVE step masks step_t = [q >= t-0.5] (q = y*mask in
  bf16) for thresholds t = 1..27.  Per-label sums are differences of adjacent
  cumulative sums, recovered on the host (linear post-processing).

  Step masks are produced on three engines concurrently:
    - DVE (t=1..18): tensor_scalar(is_ge, mult x14392) writing uint16 0x3838,
      i.e. two duplicated fp8e4m3(1.0) bytes per element -- all operands are
      2-byte so the op runs in the DVE 4x perf mode.  The PE reads the same
      bytes as fp8 through a stride-2 access pattern.
    - GpSimd (t=19..22): direct fp8 is_ge masks (also computes q = y*mask).
    - Act  (t=23..27): Sign activation -> +-1-coded fp8 masks, decoded on the
      host with analytic totals.

  The PE runs float8e4 matmuls in DoubleRow perf mode: one stream contracts a
  256-row block pair (0.5 cycles per output column, 4x the bf16 rate).  Per
  threshold the stationary weights cover 4 moments x 27 thresholds = 108 PSUM
  rows: count, k&15, k>>4 (row-in-block, split for fp8 exactness), and the
  row-block index r (ysum offset).  PSUM accumulates all 4 block pairs; one
  [108, 4096] fp32 table per core is drained at the end and reduced to the
  scalar loss on the host.
"""

import numpy as np

try:
    import concourse.bass as bass
except ImportError:  # grading env may not have trn_rl_repo on sys.path
    import sys

    sys.path.insert(0, "/opt/trn_rl_repo")
    import concourse.bass as bass

import concourse.bacc as bacc
import concourse.mybir as mybir
from concourse.tile import TileContext
from concourse.bass_utils import run_bass_kernel_spmd
from contextlib import ExitStack

fp32 = mybir.dt.float32
bf16 = mybir.dt.bfloat16
fp8 = mybir.dt.float8e4
u16 = mybir.dt.uint16

B, H, W = 32, 1024, 1024
N_CORES = 8
B_LOC = B // N_CORES  # 4 batches per core
P = 128
TWO = 2  # row blocks per matmul stream (DoubleRow contraction = 256)
NPAIR = H // (P * TWO)  # 4 row-block pairs
FW = B_LOC * W  # 4096 free columns (batch-major)
NT = 27  # thresholds 1..27
M = 4 * NT  # used PSUM rows: cnt | k&15 | k>>4 | r
M_PAD = 112  # padded to a multiple of 16 (dual-fp8 LDWEIGHTS stride rule)
CHUNK = 512  # PSUM bank width in fp32
NCHUNK = FW // CHUNK
MASK_SCALE = 14392.0  # uint16 0x3838 = two fp8e4m3(1.0) bytes

DVE_T = list(range(1, 19))
GP_T = list(range(19, 23))
ACT_T = list(range(23, 28))

L = 64  # reference label-table size


def _mean_dist_table():
    md = np.full(L, 14.0, dtype=np.float32)
    dists = {2: 18, 3: 18, 4: 18.5, 5: 19, 6: 19.5, 7: 20, 8: 20, 9: 20,
             10: 20.5, 11: 21, 12: 21.5, 13: 22, 14: 22.5, 15: 23, 16: 24.5,
             17: 24.5, 18: 26.5, 19: 28.5, 20: 29.5, 21: 33, 22: 33, 23: 33,
             24: 33, 25: 33, 26: 33}
    for k, v in dists.items():
        md[k] = v
    md[27:] = 30.0
    return md


MEAN_DIST = _mean_dist_table()


def build_weights() -> np.ndarray:
    """wc[p, k, t, i, m] fp8: per threshold t columns {t-1: 1, 27+t-1: k&15,
    54+t-1: k>>4, 81+t-1: r = 2p+i} (all values fp8e4m3-exact)."""
    import ml_dtypes

    wts = np.zeros((NPAIR, P, NT, TWO, M_PAD), ml_dtypes.float8_e4m3fn)
    k = np.arange(P)
    klo = (k & 15).astype(np.float32)
    khi = (k >> 4).astype(np.float32)
    for p in range(NPAIR):
        for ti in range(NT):
            wts[p, :, ti, :, ti] = 1.0
            wts[p, :, ti, 0, NT + ti] = klo
            wts[p, :, ti, 1, NT + ti] = klo
            wts[p, :, ti, 0, 2 * NT + ti] = khi
            wts[p, :, ti, 1, 2 * NT + ti] = khi
            wts[p, :, ti, 0, 3 * NT + ti] = float(2 * p)
            wts[p, :, ti, 1, 3 * NT + ti] = float(2 * p + 1)
    return wts


def _pe_order():
    """Interleave thresholds by estimated mask-tile completion time so the PE
    consumes each tile shortly after production (keeps pools at bufs=2)."""
    ev = []
    for i, t in enumerate(DVE_T):
        ev.append((7.0 + 2.13 * i, t))
    for j, t in enumerate(GP_T):
        ev.append((13.7 + 6.83 * j, t))
    for j, t in enumerate(ACT_T):
        ev.append((10.2 + 6.83 * j, t))
    return [t for _, t in sorted(ev)]


PE_ORDER = _pe_order()


def build_nc() -> bass.Bass:
    nc = bacc.Bacc(trn_type="TRN2")
    y = nc.dram_tensor("y", [B_LOC, H, W], fp32, kind="ExternalInput")
    m = nc.dram_tensor("m", [B_LOC, H, W], fp32, kind="ExternalInput")
    wc = nc.dram_tensor("wc", [NPAIR, P, NT, TWO, M_PAD], fp8, kind="ExternalInput")
    col_out = nc.dram_tensor("colfull", [M_PAD, FW], fp32, kind="ExternalOutput")

    with TileContext(nc) as tc, ExitStack() as ctx:
        io = ctx.enter_context(tc.tile_pool(name="io", bufs=2))
        qpool = ctx.enter_context(tc.tile_pool(name="qpool", bufs=2))
        dpool = ctx.enter_context(tc.tile_pool(name="dpool", bufs=2))
        gpool = ctx.enter_context(tc.tile_pool(name="gpool", bufs=2))
        apool = ctx.enter_context(tc.tile_pool(name="apool", bufs=2))
        wpool = ctx.enter_context(tc.tile_pool(name="wpool", bufs=2))
        cpool = ctx.enter_context(tc.tile_pool(name="cpool", bufs=1))
        psum = ctx.enter_context(tc.tile_pool(name="psum", bufs=1, space="PSUM"))

        bias = {}
        for t in ACT_T:
            bt = cpool.tile([P, 1], fp32, name=f"bias{t}")
            nc.gpsimd.memset(bt[:], 0.5 - float(t))
            bias[t] = bt

        ps = [psum.tile([M_PAD, CHUNK], fp32, name=f"ps{c}") for c in range(NCHUNK)]

        for p in range(NPAIR):
            wt = wpool.tile([P, NT, TWO, M_PAD], fp8, name="wt", tag="wt")
            nc.sync.dma_start(wt[:], wc[p])
            q = qpool.tile([P, TWO, FW], bf16, name="q", tag="q")
            for h in range(TWO):
                r = 2 * p + h
                yt = io.tile([P, FW], fp32, name="yt", tag="yt")
                mt = io.tile([P, FW], fp32, name="mt", tag="mt")
                nc.sync.dma_start(
                    yt[:], y[:, r * P : (r + 1) * P, :].rearrange("b p w -> p b w")
                )
                nc.sync.dma_start(
                    mt[:], m[:, r * P : (r + 1) * P, :].rearrange("b p w -> p b w")
                )
                nc.gpsimd.tensor_tensor(
                    q[:, h, :], yt[:], mt[:], mybir.AluOpType.mult
                )

            tiles = {}
            for t in DVE_T:
                mk = dpool.tile([P, TWO, FW, 2], fp8, name=f"d{t}", tag="d")
                for h in range(TWO):
                    nc.vector.tensor_scalar(
                        mk[:, h, :, :].bitcast(u16),
                        q[:, h, :],
                        float(t) - 0.5,
                        MASK_SCALE,
                        mybir.AluOpType.is_ge,
                        mybir.AluOpType.mult,
                    )
                tiles[t] = mk
            for t in GP_T:
                mk = gpool.tile([P, TWO, FW], fp8, name=f"g{t}", tag="g")
                for h in range(TWO):
                    nc.gpsimd.tensor_scalar(
                        mk[:, h, :], q[:, h, :], float(t) - 0.5, None,
                        mybir.AluOpType.is_ge,
                    )
                tiles[t] = mk
            for t in ACT_T:
                mk = apool.tile([P, TWO, FW], fp8, name=f"a{t}", tag="a")
                for h in range(TWO):
                    nc.scalar.activation(
                        mk[:, h, :], q[:, h, :],
                        mybir.ActivationFunctionType.Sign,
                        bias=bias[t][:], scale=1.0,
                    )
                tiles[t] = mk

            for oi, t in enumerate(PE_ORDER):
                mk = tiles[t]
                for c in range(NCHUNK):
                    cs, ce = c * CHUNK, (c + 1) * CHUNK
                    if t in DVE_T:
                        rhs = mk[:, :, cs:ce, 0]
                    else:
                        rhs = mk[:, :, cs:ce]
                    nc.tensor.matmul(
                        ps[c][:, :],
                        wt[:, t - 1, :, :],
                        rhs,
                        start=(p == 0 and oi == 0),
                        stop=(p == NPAIR - 1 and oi == NT - 1),
                        perf_mode=mybir.MatmulPerfMode.DoubleRow,
                    )

        for c in range(NCHUNK):
            drain = cpool.tile([M_PAD, CHUNK], fp32, name=f"drain{c}")
            nc.vector.tensor_copy(drain[:], ps[c][:, :])
            nc.sync.dma_start(col_out[:, c * CHUNK : (c + 1) * CHUNK], drain[:])
    nc.finalize()
    return nc


_NC = None


def _get_nc():
    global _NC
    if _NC is None:
        _NC = build_nc()
    return _NC


# analytic totals for +-1 (Sign) decode: T = sum of weights over all pixels
T_CNT = float(H)  # per column
T_KLO = 8.0 * 960.0
T_KHI = 8.0 * 448.0
T_R = 128.0 * 28.0


def finalize(colfulls):
    """Reduce per-core cumulative tables to the scalar loss."""
    counts = np.zeros((B, L), np.float64)
    ysum = np.zeros((B, L), np.float64)
    xsum = np.zeros((B, L), np.float64)
    warange = np.arange(W, dtype=np.float64)
    for c in range(N_CORES):
        cf = colfulls[c].astype(np.float64).reshape(M_PAD, B_LOC, W)
        cnt = np.zeros((NT + 1, B_LOC, W))
        klo = np.zeros((NT + 1, B_LOC))
        khi = np.zeros((NT + 1, B_LOC))
        rr = np.zeros((NT + 1, B_LOC))
        for t in range(1, NT + 1):
            crow = cf[t - 1]
            lrow = cf[NT + t - 1].sum(-1)
            hrow = cf[2 * NT + t - 1].sum(-1)
            rrow = cf[3 * NT + t - 1].sum(-1)
            if t in ACT_T:
                crow = (crow + T_CNT) / 2.0
                lrow = (lrow + T_KLO * W) / 2.0
                hrow = (hrow + T_KHI * W) / 2.0
                rrow = (rrow + T_R * W) / 2.0
            cnt[t - 1] = crow
            klo[t - 1] = lrow
            khi[t - 1] = hrow
            rr[t - 1] = rrow
        dcnt = cnt[:-1] - cnt[1:]
        dklo = klo[:-1] - klo[1:]
        dkhi = khi[:-1] - khi[1:]
        drr = rr[:-1] - rr[1:]
        for bl in range(B_LOC):
            b = c * B_LOC + bl
            counts[b, 1 : NT + 1] = dcnt[:, bl].sum(-1)
            xsum[b, 1 : NT + 1] = (dcnt[:, bl] * warange[None, :]).sum(-1)
            ysum[b, 1 : NT + 1] = (
                dklo[:, bl] + 16.0 * dkhi[:, bl] + 128.0 * drr[:, bl]
            )
    safe = np.maximum(counts, 1.0)
    yc = ysum / safe
    xc = xsum / safe
    present = counts > 0.5
    present[:, 0] = False
    pair_ok = present[:, 1:] & present[:, :-1]
    dist = np.sqrt((xc[:, 1:] - xc[:, :-1]) ** 2 + (yc[:, 1:] - yc[:, :-1]) ** 2)
    loss = np.where(pair_ok, np.abs(dist - MEAN_DIST[1:][None, :]), 0.0).sum()
    return np.float32(loss)


_WC = None


def kernel(y_pr: np.ndarray, mask: np.ndarray, _trace=False, _trace_kwargs=None):
    global _WC
    y = np.ascontiguousarray(np.asarray(y_pr, dtype=np.float32).reshape(B, H, W))
    m = np.ascontiguousarray(np.asarray(mask, dtype=np.float32))
    if _WC is None:
        _WC = build_weights()
    nc = _get_nc()
    in_maps = [
        {
            "y": y[c * B_LOC : (c + 1) * B_LOC],
            "m": m[c * B_LOC : (c + 1) * B_LOC],
            "wc": _WC,
        }
        for c in range(N_CORES)
    ]
    kw = {}
    if _trace:
        kw["trace"] = True
        kw.update(_trace_kwargs or {})
    res = run_bass_kernel_spmd(nc, in_maps, core_ids=list(range(N_CORES)), **kw)
    loss = finalize([r["colfull"] for r in res.results])
    if _trace:
        return loss, res
    return loss
